# revision 16
# baseline (speedup 1.0000x reference)
"""Trainium2 Bass kernel for Conv2d_NN (k-NN gather + grouped conv1d).

Shapes (hardcoded): x (32, 32, 96, 96) f32, conv_w (256, 128, 9) f32,
conv_b (256,) f32 -> out (32, 64, 96, 96) f32.

Strategy: data-parallel over batch across 8 NeuronCores (4 batches/core).
Per batch on device (tokens N=2304, features D=128 after host pixel-unshuffle):
  - scores = x2^T @ x2 in fp32 on PE, PSUM-chunked [128,512] (self-match
    excluded with a -BIG*I diagonal matmul); ACT evacuates each chunk
  - the j-dependent -0.5*||x_j||^2 term is folded in as a 1-row augmented
    matmul closing each chunk's accumulation group (top-k ranking needs
    fp32-exact scores; gpsimd adds would eat its ~10us semaphore latency
    per tile, PE rows are cheaper)
  - DVE max8 / find_index8 give each token's top-8 neighbors
  - a 2-hop DMA shuffle builds the wrapped index layout, one gpsimd
    ap_gather per 4-tile group gathers neighbor feature columns
  - conv1d = 9 accumulating 128x128 matmuls per output half in bf16
    (4x faster than fp32; bf16 noise ~3e-3 is fine for the conv, fatal for
    the distance ranking, which is why scores stay fp32); gathered features
    are cast to bf16 on ACT; ACT adds bias + ReLU; DMA writes (b, 256, 2304)
  - the whole thing is software-pipelined: conv for group g is emitted
    several tiles later so the shuffle/gather latency hides behind the
    DVE-paced tile stream.
Host does pixel-unshuffle/shuffle.
"""

import sys

for _p in ("/opt/trn_rl_repo",):
    if _p not in sys.path:
        sys.path.insert(0, _p)

import numpy as np

import concourse.bass as bass
import concourse.mybir as mybir
import concourse.tile as tile
from concourse import bacc, bass_utils

# Problem constants
B, C_IN, C_OUT, H, W = 32, 32, 64, 96, 96
S = 2
K = 9
D = C_IN * S * S            # 128
D_OUT = C_OUT * S * S       # 256
N = (H // S) * (W // S)     # 2304
NCORES = 8
BPC = B // NCORES           # 4 batches per core

P = 128                     # partitions / m-tile size
NT = N // P                 # 18 m-tiles
CHUNK = 512                 # psum bank = 512 f32
CHUNKS = [(c, min(CHUNK, N - c)) for c in range(0, N, CHUNK)]  # 4x512 + 256
BIG = 1.0e30
GROUP_TILES = 4             # m-tiles per conv group (512 tokens)

# pipeline lags (in tile slots)
GATHER_LAG = 3              # gather for tile t emitted at slot t+GATHER_LAG
CONV_LAG = 8                # conv for group ending at tile t emitted at slot t+CONV_LAG
BSTART_LEAD = 6             # batch-start work emitted this many slots early

_cache = {}


def _build_kernel(bpc=BPC, nt=NT):
    key = ("nc", bpc, nt)
    if key in _cache:
        return _cache[key], None

    nc = bacc.Bacc("TRN2", target_bir_lowering=False, debug=False)

    f32 = mybir.dt.float32
    bf16 = mybir.dt.bfloat16
    u16 = mybir.dt.uint16
    i16 = mybir.dt.int16

    # I/O
    x2_d = nc.dram_tensor("x2", [bpc, D, N], f32, kind="ExternalInput")
    wt_d = nc.dram_tensor("wt", [D, K, 2, P], bf16, kind="ExternalInput")
    bias_d = nc.dram_tensor("bias", [P, 2], f32, kind="ExternalInput")
    ident_d = nc.dram_tensor("ident", [P, P], f32, kind="ExternalInput")
    negbig_d = nc.dram_tensor("negbig", [P, P], f32, kind="ExternalInput")
    neghalf_d = nc.dram_tensor("neghalf", [P, 1], f32, kind="ExternalInput")
    ones1_d = nc.dram_tensor("ones1", [1, P], f32, kind="ExternalInput")
    out_d = nc.dram_tensor("out", [bpc, D_OUT, N], f32, kind="ExternalOutput")

    # groups per batch: (start_tile, n_tiles)
    groups = []
    mt = 0
    while mt < nt:
        gt = min(GROUP_TILES, nt - mt)
        groups.append((mt, gt))
        mt += gt

    T = bpc * nt  # total tile slots

    with tile.TileContext(nc) as tc:
        import contextlib

        with contextlib.ExitStack() as ctx:
            const_pool = ctx.enter_context(tc.tile_pool(name="consts", bufs=1))
            x2_pool = ctx.enter_context(tc.tile_pool(name="x2", bufs=2))
            sq_pool = ctx.enter_context(tc.tile_pool(name="sq", bufs=1))
            nsqh_pool = ctx.enter_context(tc.tile_pool(name="nsqh", bufs=2))
            scores_pool = ctx.enter_context(tc.tile_pool(name="scores", bufs=4))
            mx_pool = ctx.enter_context(tc.tile_pool(name="mx", bufs=4))
            widx_pool = ctx.enter_context(tc.tile_pool(name="widx", bufs=3))
            g_pool = ctx.enter_context(tc.tile_pool(name="g", bufs=3))
            outs_pool = ctx.enter_context(tc.tile_pool(name="outs", bufs=4))
            psum_sc_pool = ctx.enter_context(
                tc.tile_pool(name="psums", bufs=3, space="PSUM")
            )
            psum_conv_pool = ctx.enter_context(
                tc.tile_pool(name="psumc", bufs=2, space="PSUM")
            )
            psum_nsq_pool = ctx.enter_context(
                tc.tile_pool(name="psumn", bufs=1, space="PSUM")
            )
            dram_pool = ctx.enter_context(
                tc.tile_pool(name="stage", bufs=4, space="DRAM")
            )

            # constants, loaded once (weights in bf16 for fast conv matmuls)
            wt_s = const_pool.tile([D, K * 2 * P], bf16, tag="wt")
            nc.sync.dma_start(
                wt_s[:], wt_d.ap().rearrange("d k h c -> d (k h c)")
            )
            wt_v = wt_s[:].rearrange("d (k h c) -> d k h c", k=K, h=2, c=P)
            bias_s = const_pool.tile([P, 2], f32, tag="bias")
            nc.sync.dma_start(bias_s[:], bias_d.ap())
            ident_s = const_pool.tile([P, P], f32, tag="ident")
            nc.sync.dma_start(ident_s[:], ident_d.ap())
            negbig_s = const_pool.tile([P, P], f32, tag="negbig")
            nc.sync.dma_start(negbig_s[:], negbig_d.ap())
            neghalf_s = const_pool.tile([P, 1], f32, tag="neghalf")
            nc.sync.dma_start(neghalf_s[:], neghalf_d.ap())
            ones1_s = const_pool.tile([1, P], f32, tag="ones1")
            nc.sync.dma_start(ones1_s[:], ones1_d.ap())

            # per-batch state (rotating pool tiles), keyed by batch
            state = {}

            def emit_bstart(b):
                x2 = x2_pool.tile([D, N], f32, tag="x2")
                nc.sync.dma_start(x2[:], x2_d.ap()[b])
                # bf16 copy of x2 for the conv's k=0 (self) columns
                x2bf = x2_pool.tile([D, N], bf16, tag="x2bf")
                nc.scalar.copy(x2bf[:], x2[:])
                sq = sq_pool.tile([D, N], f32, tag="sq")
                nc.scalar.square(sq[:], x2[:])
                nsqh = nsqh_pool.tile([1, N], f32, tag="nsqh")
                for c0, w in CHUNKS:
                    pn = psum_nsq_pool.tile([1, CHUNK], f32, tag="pnsq")
                    nc.tensor.matmul(
                        pn[:1, :w], lhsT=neghalf_s[:], rhs=sq[:, c0 : c0 + w],
                        start=True, stop=True,
                    )
                    nc.scalar.copy(nsqh[:1, c0 : c0 + w], pn[:1, :w])
                state[b] = dict(
                    x2=x2, x2bf=x2bf, nsqh=nsqh, widx={}, g={}, scores={}, midx={}
                )

            def emit_scores(b, mt):
                st = state[b]
                x2 = st["x2"]
                nsqh = st["nsqh"]
                m0 = mt * P
                scores = scores_pool.tile([P, N], f32, tag="scores")
                diag_chunk = m0 // CHUNK
                for ci, (c0, w) in enumerate(CHUNKS):
                    psc = psum_sc_pool.tile([P, CHUNK], f32, tag="psc")
                    nc.tensor.matmul(
                        psc[:, :w],
                        lhsT=x2[:, m0 : m0 + P],
                        rhs=x2[:, c0 : c0 + w],
                        start=True, stop=False,
                    )
                    if ci == diag_chunk:
                        # self-exclusion: scores[p, m0+p] -= BIG
                        d0 = m0 - c0
                        nc.tensor.matmul(
                            psc[:, d0 : d0 + P],
                            lhsT=negbig_s[:],
                            rhs=ident_s[:],
                            start=False, stop=False,
                        )
                    # j-dependent -0.5*nsq[j] row closes the group
                    nc.tensor.matmul(
                        psc[:, :w],
                        lhsT=ones1_s[:],
                        rhs=nsqh[:1, c0 : c0 + w],
                        start=False, stop=True,
                    )
                    nc.scalar.copy(scores[:, c0 : c0 + w], psc[:, :w])
                st["scores"][mt] = scores

            def emit_topk(b, mt):
                st = state[b]
                scores = st["scores"].pop(mt)
                mx8 = mx_pool.tile([P, 8], f32, tag="mx8")
                nc.vector.max(out=mx8[:], in_=scores[:])
                midx = mx_pool.tile([P, 8], u16, tag="midx")
                nc.vector.max_index(midx[:], mx8[:], scores[:])
                st["midx"][mt] = midx

            def emit_shuffle(b, mt):
                st = state[b]
                midx = st["midx"].pop(mt)
                gidx = mt // GROUP_TILES
                gstart, gtiles = groups[gidx]
                pos = mt - gstart
                if pos == 0:
                    wg = widx_pool.tile([P, GROUP_TILES * 64], i16, tag="widx")
                    st["widx"][gidx] = wg
                wg = st["widx"][gidx]
                # hop 1: midx [128,8] -> staging[r*64 + u*8 + k] (DRAM)
                stage_t = dram_pool.tile([1, 1024], u16, tag="stage")
                st_dst = stage_t[:].rearrange(
                    "a (r u k) -> a u r k", r=16, u=8, k=8
                ).squeeze(0)
                nc.sync.dma_start(st_dst, midx[:])
                # hop 2: widx[16c+r, pos*64 + c2] = staging[r*64 + c2]
                st_src = (
                    stage_t[:]
                    .rearrange("a (r c2) -> a r c2", r=16, c2=64)
                    .unsqueeze(1)
                    .broadcast_to([1, 8, 16, 64])
                    .bitcast(i16)
                    .squeeze(0)
                )
                nc.sync.dma_start(wg[:, pos * 64 : (pos + 1) * 64], st_src)

            def emit_gather(b, gidx):
                st = state[b]
                gstart, gtiles = groups[gidx]
                wg = st["widx"].pop(gidx)
                gg = g_pool.tile([D, GROUP_TILES * 1024], f32, tag="g", bufs=2)
                nc.gpsimd.ap_gather(
                    gg[:, : gtiles * 1024],
                    st["x2"][:],
                    wg[:, : gtiles * 64],
                    channels=P,
                    num_elems=N,
                    d=1,
                    num_idxs=gtiles * 1024,
                )
                # cast gathered features to bf16 for the conv matmuls
                ggbf = g_pool.tile([D, GROUP_TILES * 1024], bf16, tag="gbf")
                nc.scalar.copy(ggbf[:, : gtiles * 1024], gg[:, : gtiles * 1024])
                st["g"][gidx] = ggbf

            def emit_conv(b, gidx):
                st = state[b]
                x2bf = st["x2bf"]
                gstart, gtiles = groups[gidx]
                ggbf = st["g"].pop(gidx)
                gtok = gtiles * P
                g0 = gstart * P
                gv = ggbf[:, : gtiles * 1024].rearrange(
                    "d (mt u k r) -> d mt u k r", mt=gtiles, u=8, k=8, r=16
                )
                for h in range(2):
                    cp = psum_conv_pool.tile([P, CHUNK], f32, tag="pconv")
                    # k = 0: self columns, no gather needed
                    nc.tensor.matmul(
                        cp[:, :gtok],
                        lhsT=wt_v[:, 0, h, :],
                        rhs=x2bf[:, g0 : g0 + gtok],
                        start=True, stop=False,
                    )
                    for k in range(1, K):
                        nc.tensor.matmul(
                            cp[:, :gtok],
                            lhsT=wt_v[:, k, h, :],
                            rhs=gv[:, :, :, k - 1, :],
                            start=False, stop=(k == K - 1),
                        )
                    o_s = outs_pool.tile([P, CHUNK], f32, tag="outs")
                    nc.scalar.activation(
                        o_s[:, :gtok], cp[:, :gtok],
                        mybir.ActivationFunctionType.Relu,
                        bias=bias_s[:, h : h + 1],
                    )
                    nc.sync.dma_start(
                        out_d.ap()[b, h * P : (h + 1) * P, g0 : g0 + gtok],
                        o_s[:, :gtok],
                    )
                if not st["g"] and gidx == len(groups) - 1:
                    state.pop(b, None)

            # ---- software-pipelined emission over flat tile slots ----
            emit_bstart(0)
            for s in range(T + CONV_LAG + 1):
                # batch-start for upcoming batch
                if s + BSTART_LEAD < T and (s + BSTART_LEAD) % nt == 0:
                    emit_bstart((s + BSTART_LEAD) // nt)
                if s < T:
                    b, mt = divmod(s, nt)
                    emit_scores(b, mt)
                    emit_topk(b, mt)
                    emit_shuffle(b, mt)
                t_g = s - GATHER_LAG
                if 0 <= t_g < T:
                    b, mt = divmod(t_g, nt)
                    gidx = mt // GROUP_TILES
                    gstart, gtiles = groups[gidx]
                    if mt == gstart + gtiles - 1:
                        emit_gather(b, gidx)
                t_c = s - CONV_LAG
                if 0 <= t_c < T:
                    b, mt = divmod(t_c, nt)
                    gidx = mt // GROUP_TILES
                    gstart, gtiles = groups[gidx]
                    if mt == gstart + gtiles - 1:
                        emit_conv(b, gidx)

    nc.compile()
    _cache[key] = nc
    return nc, None


def _host_inputs(x, conv_w, conv_b):
    """Shared per-core constant inputs + per-core x2 slices."""
    x = np.ascontiguousarray(x, dtype=np.float32)
    b = x.shape[0]
    x1 = (
        x.reshape(b, C_IN, H // S, S, W // S, S)
        .transpose(0, 1, 3, 5, 2, 4)
        .reshape(b, D, N)
    )
    import ml_dtypes

    wt = np.ascontiguousarray(
        conv_w.reshape(2, P, D, K).transpose(2, 3, 0, 1).astype(ml_dtypes.bfloat16)
    )  # [D, K, 2, P] bf16; conv_w is (256,128,9) -> (2,128half) x d x k
    bias = np.ascontiguousarray(
        conv_b.reshape(2, P).transpose(1, 0), dtype=np.float32
    )  # [P, 2]
    ident = np.eye(P, dtype=np.float32)
    negbig = (-BIG * np.eye(P)).astype(np.float32)
    neghalf = np.full((P, 1), -0.5, dtype=np.float32)
    ones1 = np.ones((1, P), dtype=np.float32)
    return x1, dict(
        wt=wt, bias=bias, ident=ident, negbig=negbig, neghalf=neghalf,
        ones1=ones1
    )


def kernel(x, conv_w, conv_b):
    nc, _ = _build_kernel()
    x1, consts = _host_inputs(x, conv_w, conv_b)
    in_maps = []
    for c in range(NCORES):
        m = dict(consts)
        m["x2"] = np.ascontiguousarray(x1[c * BPC : (c + 1) * BPC])
        in_maps.append(m)
    res = bass_utils.run_bass_kernel_spmd(nc, in_maps, core_ids=list(range(NCORES)))
    outs = np.concatenate([r["out"] for r in res.results], axis=0)  # [B, 256, N]
    # pixel shuffle back: channel dim = (co, sy, sx); token = (h, w)
    o = outs.reshape(B, C_OUT, S, S, H // S, W // S)
    o = o.transpose(0, 1, 4, 2, 5, 3).reshape(B, C_OUT, H, W)
    return np.ascontiguousarray(o, dtype=np.float32)


# revision 20
# speedup vs baseline: 1.3820x; 1.3820x over previous
"""Trainium2 Bass kernel for Conv2d_NN (k-NN gather + grouped conv1d).

Shapes (hardcoded): x (32, 32, 96, 96) f32, conv_w (256, 128, 9) f32,
conv_b (256,) f32 -> out (32, 64, 96, 96) f32.

Strategy: data-parallel over batch across 8 NeuronCores (4 batches/core).
Per batch on device (tokens N=2304, features D=128 after host pixel-unshuffle):
  - scores = x2^T @ x2 in fp32 on PE, PSUM-chunked [128,512]; self-match
    excluded with a -BIG*I diagonal matmul; the j-dependent -0.5*||x_j||^2
    term is a 1-row matmul closing each chunk's group (ranking needs
    fp32-exact scores: bf16/f32r anywhere in this path flips neighbors and
    blows the output error up by >10x)
  - ACT evacuates each PSUM chunk; DVE max8 / find_index8 give the top-8
  - the -0.5*nsq row runs as an exact fp16 hi+lo split (2 streams at
    1 cyc/col instead of fp32's 4; residual ~7e-6, far under the ~3e-5
    score-noise tolerance)
  - a 2-hop DMA shuffle builds ap_gather's wrapped index layout; one
    gpsimd ap_gather per 4-tile group pulls neighbor columns into a
    per-batch fp32 buffer
  - conv1d = 9 accumulating 128x128 fp32 matmuls per output half (bf16/f32r
    rhs would need a cast step that stalls whichever engine runs it); ACT
    adds bias + ReLU; DMA writes (b, 256, 2304)
  - conv runs half a batch behind the score/topk stream: the gather/
    shuffle chain has 10s of us of DMA-completion + gpsimd-wakeup latency,
    and ~45us of slack keeps it off the critical path (per-group lags
    stall the whole machine).
Host does pixel-unshuffle/shuffle.
"""

import sys

for _p in ("/opt/trn_rl_repo",):
    if _p not in sys.path:
        sys.path.insert(0, _p)

import numpy as np

import concourse.bass as bass
import concourse.mybir as mybir
import concourse.tile as tile
from concourse import bacc, bass_utils

# Problem constants
B, C_IN, C_OUT, H, W = 32, 32, 64, 96, 96
S = 2
K = 9
D = C_IN * S * S            # 128
D_OUT = C_OUT * S * S       # 256
N = (H // S) * (W // S)     # 2304
NCORES = 8
BPC = B // NCORES           # 4 batches per core

P = 128                     # partitions / m-tile size
NT = N // P                 # 18 m-tiles
CHUNK = 512                 # psum bank = 512 f32
CHUNKS = [(c, min(CHUNK, N - c)) for c in range(0, N, CHUNK)]  # 4x512 + 256
BIG = 1.0e30
GROUP_TILES = 4             # m-tiles per conv group (512 tokens)

# pipeline lags (in tile slots)
GATHER_LAG = 3              # gather emitted this many slots after its group ends
BSTART_LEAD = 6             # batch-start work emitted this many slots early
CONV_EXTRA = 2              # conv trails the gather by half a batch + this

_cache = {}


def _build_kernel(bpc=BPC, nt=NT):
    key = ("nc", bpc, nt)
    if key in _cache:
        return _cache[key], None

    nc = bacc.Bacc("TRN2", target_bir_lowering=False, debug=False)

    f32 = mybir.dt.float32
    fp16 = mybir.dt.float16
    u16 = mybir.dt.uint16
    i16 = mybir.dt.int16

    # I/O
    x2_d = nc.dram_tensor("x2", [bpc, D, N], f32, kind="ExternalInput")
    wt_d = nc.dram_tensor("wt", [D, K, 2, P], f32, kind="ExternalInput")
    bias_d = nc.dram_tensor("bias", [P, 2], f32, kind="ExternalInput")
    ident_d = nc.dram_tensor("ident", [P, P], f32, kind="ExternalInput")
    negbig_d = nc.dram_tensor("negbig", [P, P], f32, kind="ExternalInput")
    neghalf_d = nc.dram_tensor("neghalf", [P, 1], f32, kind="ExternalInput")
    ones1_d = nc.dram_tensor("ones1", [1, P], f32, kind="ExternalInput")
    ones16_d = nc.dram_tensor("ones16", [1, P], fp16, kind="ExternalInput")
    out_d = nc.dram_tensor("out", [bpc, D_OUT, N], f32, kind="ExternalOutput")

    # groups per batch: (start_tile, n_tiles)
    groups = []
    mt = 0
    while mt < nt:
        gt = min(GROUP_TILES, nt - mt)
        groups.append((mt, gt))
        mt += gt

    T = bpc * nt                                     # total tile slots
    CONV_DELAY = nt // 2 + GATHER_LAG + CONV_EXTRA   # conv trails group end by this

    with tile.TileContext(nc) as tc:
        import contextlib

        with contextlib.ExitStack() as ctx:
            const_pool = ctx.enter_context(tc.tile_pool(name="consts", bufs=1))
            x2_pool = ctx.enter_context(tc.tile_pool(name="x2", bufs=2))
            sq_pool = ctx.enter_context(tc.tile_pool(name="sq", bufs=2))
            nsqh_pool = ctx.enter_context(tc.tile_pool(name="nsqh", bufs=2))
            scores_pool = ctx.enter_context(tc.tile_pool(name="scores", bufs=3))
            mx_pool = ctx.enter_context(tc.tile_pool(name="mx", bufs=4))
            widx_pool = ctx.enter_context(tc.tile_pool(name="widx", bufs=3))
            g_pool = ctx.enter_context(tc.tile_pool(name="g", bufs=4))
            outs_pool = ctx.enter_context(tc.tile_pool(name="outs", bufs=4))
            psum_sc_pool = ctx.enter_context(
                tc.tile_pool(name="psums", bufs=3, space="PSUM")
            )
            psum_conv_pool = ctx.enter_context(
                tc.tile_pool(name="psumc", bufs=2, space="PSUM")
            )
            psum_nsq_pool = ctx.enter_context(
                tc.tile_pool(name="psumn", bufs=1, space="PSUM")
            )
            dram_pool = ctx.enter_context(
                tc.tile_pool(name="stage", bufs=4, space="DRAM")
            )

            # constants, loaded once
            wt_s = const_pool.tile([D, K * 2 * P], f32, tag="wt")
            nc.sync.dma_start(wt_s[:], wt_d.ap().rearrange("d k h c -> d (k h c)"))
            wt_v = wt_s[:].rearrange("d (k h c) -> d k h c", k=K, h=2, c=P)
            bias_s = const_pool.tile([P, 2], f32, tag="bias")
            nc.sync.dma_start(bias_s[:], bias_d.ap())
            ident_s = const_pool.tile([P, P], f32, tag="ident")
            nc.sync.dma_start(ident_s[:], ident_d.ap())
            negbig_s = const_pool.tile([P, P], f32, tag="negbig")
            nc.sync.dma_start(negbig_s[:], negbig_d.ap())
            neghalf_s = const_pool.tile([P, 1], f32, tag="neghalf")
            nc.sync.dma_start(neghalf_s[:], neghalf_d.ap())
            ones1_s = const_pool.tile([1, P], f32, tag="ones1")
            nc.sync.dma_start(ones1_s[:], ones1_d.ap())
            ones16_s = const_pool.tile([1, P], fp16, tag="ones16")
            nc.sync.dma_start(ones16_s[:], ones16_d.ap())

            # per-batch state (rotating pool tiles), keyed by batch
            state = {}

            def emit_bstart(b):
                x2 = x2_pool.tile([D, N], f32, tag="x2")
                nc.sync.dma_start(x2[:], x2_d.ap()[b])
                nsqh = nsqh_pool.tile([1, N], f32, tag="nsqh")
                for c0, w in CHUNKS:
                    sq = sq_pool.tile([D, CHUNK], f32, tag="sq")
                    nc.scalar.square(sq[:, :w], x2[:, c0 : c0 + w])
                    pn = psum_nsq_pool.tile([1, CHUNK], f32, tag="pnsq")
                    nc.tensor.matmul(
                        pn[:1, :w], lhsT=neghalf_s[:], rhs=sq[:, :w],
                        start=True, stop=True,
                    )
                    nc.scalar.copy(nsqh[:1, c0 : c0 + w], pn[:1, :w])
                # exact fp16 hi+lo split of the nsq row (2x faster PE stream)
                nsq_hi = nsqh_pool.tile([1, N], fp16, tag="nsqhi")
                nc.scalar.copy(nsq_hi[:], nsqh[:])
                nsq_hi32 = nsqh_pool.tile([1, N], f32, tag="nsqhi32")
                nc.scalar.copy(nsq_hi32[:], nsq_hi[:])
                nsq_lo32 = nsqh_pool.tile([1, N], f32, tag="nsqlo32")
                nc.gpsimd.tensor_sub(nsq_lo32[:], nsqh[:], nsq_hi32[:])
                nsq_lo = nsqh_pool.tile([1, N], fp16, tag="nsqlo")
                nc.scalar.copy(nsq_lo[:], nsq_lo32[:])
                state[b] = dict(
                    x2=x2, nsq_hi=nsq_hi, nsq_lo=nsq_lo, g={},
                    widx={}, scores={}, midx={},
                )

            def emit_scores(b, mt):
                st = state[b]
                x2 = st["x2"]
                nsq_hi = st["nsq_hi"]
                nsq_lo = st["nsq_lo"]
                m0 = mt * P
                scores = scores_pool.tile([P, N], f32, tag="scores")
                diag_chunk = m0 // CHUNK
                for ci, (c0, w) in enumerate(CHUNKS):
                    psc = psum_sc_pool.tile([P, CHUNK], f32, tag="psc")
                    nc.tensor.matmul(
                        psc[:, :w],
                        lhsT=x2[:, m0 : m0 + P],
                        rhs=x2[:, c0 : c0 + w],
                        start=True, stop=False,
                    )
                    if ci == diag_chunk:
                        # self-exclusion: scores[p, m0+p] -= BIG
                        d0 = m0 - c0
                        nc.tensor.matmul(
                            psc[:, d0 : d0 + P],
                            lhsT=negbig_s[:],
                            rhs=ident_s[:],
                            start=False, stop=False,
                        )
                    # j-dependent -0.5*nsq[j] rows (fp16 hi+lo) close the group
                    nc.tensor.matmul(
                        psc[:, :w],
                        lhsT=ones16_s[:],
                        rhs=nsq_hi[:1, c0 : c0 + w],
                        start=False, stop=False,
                    )
                    nc.tensor.matmul(
                        psc[:, :w],
                        lhsT=ones16_s[:],
                        rhs=nsq_lo[:1, c0 : c0 + w],
                        start=False, stop=True,
                    )
                    nc.scalar.copy(scores[:, c0 : c0 + w], psc[:, :w])
                st["scores"][mt] = scores

            def emit_topk(b, mt):
                st = state[b]
                scores = st["scores"].pop(mt)
                mx8 = mx_pool.tile([P, 8], f32, tag="mx8")
                nc.vector.max(out=mx8[:], in_=scores[:])
                midx = mx_pool.tile([P, 8], u16, tag="midx")
                nc.vector.max_index(midx[:], mx8[:], scores[:])
                st["midx"][mt] = midx

            def emit_shuffle(b, mt):
                st = state[b]
                midx = st["midx"].pop(mt)
                gidx = mt // GROUP_TILES
                gstart, gtiles = groups[gidx]
                pos = mt - gstart
                if pos == 0:
                    wg = widx_pool.tile([P, GROUP_TILES * 64], i16, tag="widx")
                    st["widx"][gidx] = wg
                wg = st["widx"][gidx]
                # hop 1: midx [128,8] -> staging[r*64 + u*8 + k] (DRAM)
                stage_t = dram_pool.tile([1, 1024], u16, tag="stage")
                st_dst = stage_t[:].rearrange(
                    "a (r u k) -> a u r k", r=16, u=8, k=8
                ).squeeze(0)
                nc.sync.dma_start(st_dst, midx[:])
                # hop 2: widx[16c+r, pos*64 + c2] = staging[r*64 + c2]
                st_src = (
                    stage_t[:]
                    .rearrange("a (r c2) -> a r c2", r=16, c2=64)
                    .unsqueeze(1)
                    .broadcast_to([1, 8, 16, 64])
                    .bitcast(i16)
                    .squeeze(0)
                )
                nc.sync.dma_start(wg[:, pos * 64 : (pos + 1) * 64], st_src)

            def emit_gather(b, gidx):
                st = state[b]
                gstart, gtiles = groups[gidx]
                wg = st["widx"].pop(gidx)
                gg = g_pool.tile([D, GROUP_TILES * 1024], f32, tag="g")
                nc.gpsimd.ap_gather(
                    gg[:, : gtiles * 1024],
                    st["x2"][:],
                    wg[:, : gtiles * 64],
                    channels=P,
                    num_elems=N,
                    d=1,
                    num_idxs=gtiles * 1024,
                )
                st["g"][gidx] = gg

            def emit_conv(b, gidx):
                st = state[b]
                x2 = st["x2"]
                gstart, gtiles = groups[gidx]
                gtok = gtiles * P
                g0 = gstart * P
                gg = st["g"].pop(gidx)
                gv = gg[:, : gtiles * 1024].rearrange(
                    "d (mt u k r) -> d mt u k r", mt=gtiles, u=8, k=8, r=16
                )
                for h in range(2):
                    cp = psum_conv_pool.tile([P, CHUNK], f32, tag="pconv")
                    # k = 0: self columns, no gather needed
                    nc.tensor.matmul(
                        cp[:, :gtok],
                        lhsT=wt_v[:, 0, h, :],
                        rhs=x2[:, g0 : g0 + gtok],
                        start=True, stop=False,
                    )
                    for k in range(1, K):
                        nc.tensor.matmul(
                            cp[:, :gtok],
                            lhsT=wt_v[:, k, h, :],
                            rhs=gv[:, :, :, k - 1, :],
                            start=False, stop=(k == K - 1),
                        )
                    o_s = outs_pool.tile([P, CHUNK], f32, tag="outs")
                    nc.scalar.activation(
                        o_s[:, :gtok], cp[:, :gtok],
                        mybir.ActivationFunctionType.Relu,
                        bias=bias_s[:, h : h + 1],
                    )
                    nc.sync.dma_start(
                        out_d.ap()[b, h * P : (h + 1) * P, g0 : g0 + gtok],
                        o_s[:, :gtok],
                    )
                if gidx == len(groups) - 1:
                    state.pop(b, None)

            # ---- software-pipelined emission over flat tile slots ----
            emit_bstart(0)
            for s in range(T + CONV_DELAY + 1):
                if s + BSTART_LEAD < T and (s + BSTART_LEAD) % nt == 0:
                    emit_bstart((s + BSTART_LEAD) // nt)
                if s < T:
                    b, mt = divmod(s, nt)
                    emit_scores(b, mt)
                    emit_topk(b, mt)
                    emit_shuffle(b, mt)
                t_g = s - GATHER_LAG
                if 0 <= t_g < T:
                    b, mt = divmod(t_g, nt)
                    gidx = mt // GROUP_TILES
                    gstart, gtiles = groups[gidx]
                    if mt == gstart + gtiles - 1:
                        emit_gather(b, gidx)
                t_c = s - CONV_DELAY
                if 0 <= t_c < T:
                    b, mt = divmod(t_c, nt)
                    gidx = mt // GROUP_TILES
                    gstart, gtiles = groups[gidx]
                    if mt == gstart + gtiles - 1:
                        emit_conv(b, gidx)

    nc.compile()
    _cache[key] = nc
    return nc, None


def _host_inputs(x, conv_w, conv_b):
    """Shared per-core constant inputs + per-core x2 slices."""
    import ml_dtypes

    x = np.ascontiguousarray(x, dtype=np.float32)
    b = x.shape[0]
    x1 = (
        x.reshape(b, C_IN, H // S, S, W // S, S)
        .transpose(0, 1, 3, 5, 2, 4)
        .reshape(b, D, N)
    )
    wt = np.ascontiguousarray(
        conv_w.reshape(2, P, D, K).transpose(2, 3, 0, 1), dtype=np.float32
    )  # [D, K, 2, P]; conv_w is (256,128,9) -> (2,128half) x d x k
    bias = np.ascontiguousarray(
        conv_b.reshape(2, P).transpose(1, 0), dtype=np.float32
    )  # [P, 2]
    ident = np.eye(P, dtype=np.float32)
    negbig = (-BIG * np.eye(P)).astype(np.float32)
    neghalf = np.full((P, 1), -0.5, dtype=np.float32)
    ones1 = np.ones((1, P), dtype=np.float32)
    ones16 = np.ones((1, P), dtype=np.float16)
    return x1, dict(
        wt=wt, bias=bias, ident=ident, negbig=negbig, neghalf=neghalf,
        ones1=ones1, ones16=ones16
    )


def kernel(x, conv_w, conv_b):
    nc, _ = _build_kernel()
    x1, consts = _host_inputs(x, conv_w, conv_b)
    in_maps = []
    for c in range(NCORES):
        m = dict(consts)
        m["x2"] = np.ascontiguousarray(x1[c * BPC : (c + 1) * BPC])
        in_maps.append(m)
    res = bass_utils.run_bass_kernel_spmd(nc, in_maps, core_ids=list(range(NCORES)))
    outs = np.concatenate([r["out"] for r in res.results], axis=0)  # [B, 256, N]
    # pixel shuffle back: channel dim = (co, sy, sx); token = (h, w)
    o = outs.reshape(B, C_OUT, S, S, H // S, W // S)
    o = o.transpose(0, 1, 4, 2, 5, 3).reshape(B, C_OUT, H, W)
    return np.ascontiguousarray(o, dtype=np.float32)
